# revision 11
# baseline (speedup 1.0000x reference)
"""Trainium2 Bass kernel for DebiasedAttention.

Math:
  scores = (Q @ K^T) * scale * t_i * t_j          (bias folded into Q,K rows)
  scores = where(mask==0, -1e9, scores)           (mask injected additively via PE)
  p      = softmax(scores)   (no max-subtraction needed: |scores| <= ~40)
  out    = p @ V

Sharding: batch b -> core b (8 cores, zero communication).

Per-core pipeline (H=8 heads, S=2048, D=32):
  PE   : S_psum = (NEG*I) @ (1-mask)  then  += qT'^T @ kT'  (fp32r, full rate)
  ACT  : e = exp(S_psum) -> SBUF, accum_out = rowsum r      (masked entries exp -> 0)
  DVE  : rinv = 1/r ; cast p -> fp16 ; copy pT PSUM->SBUF
  GPSIMD: p = e * rinv   (the 1-GiB output tensor)
  PE   : pT = transpose(p_fp16) ; out^T += V_chunk^T @ pT_chunk ; final transpose
  DMA  : p (128 MiB/core) + out (2 MiB/core) to HBM
"""

import math
from contextlib import ExitStack

import numpy as np

B, H, S, D = 8, 8, 2048, 32
NCORES = 8
NIB = S // 128          # 16 row blocks
SCALE = 1.0 / math.sqrt(D)
NEG = -60000.0          # additive mask value; exp(NEG + s) == 0.0 exactly in fp32

_cache = {}


def _build_nc():
    import concourse.bass as bass  # noqa: F401
    import concourse.mybir as mybir
    import concourse.tile as tile
    from concourse import bacc

    f32 = mybir.dt.float32
    f32r = mybir.dt.float32r
    f16 = mybir.dt.float16
    i32 = mybir.dt.int32
    Exp = mybir.ActivationFunctionType.Exp
    MULT = mybir.AluOpType.mult
    ADD = mybir.AluOpType.add
    NEQ = mybir.AluOpType.not_equal

    nc = bacc.Bacc(
        "TRN2",
        target_bir_lowering=False,
        debug=False,
        enable_asserts=False,
        num_devices=NCORES,
    )

    q_d = nc.dram_tensor("q", (H, S, D), f32, kind="ExternalInput").ap()
    k_d = nc.dram_tensor("k", (H, S, D), f32, kind="ExternalInput").ap()
    v_d = nc.dram_tensor("v", (H, S, D), f32, kind="ExternalInput").ap()
    t_d = nc.dram_tensor("t", (S,), f32, kind="ExternalInput").ap()
    m_d = nc.dram_tensor("m", (S, S), i32, kind="ExternalInput").ap()
    p_d = nc.dram_tensor("p", (H, S, S), f32, kind="ExternalOutput").ap()
    o_d = nc.dram_tensor("o", (H, S, D), f32, kind="ExternalOutput").ap()

    with tile.TileContext(nc) as tc, ExitStack() as ctx:
        consts = ctx.enter_context(tc.tile_pool(name="consts", bufs=1))

        # identity matrices for PE transposes
        identF = consts.tile([128, 128], f16)
        nc.gpsimd.memset(identF[:], 0.0)
        nc.gpsimd.affine_select(
            out=identF[:], in_=identF[:], compare_op=NEQ, fill=1.0,
            base=0, pattern=[[-1, 128]], channel_multiplier=1,
        )
        ident32 = consts.tile([32, 32], f32)
        nc.gpsimd.memset(ident32[:], 0.0)
        nc.gpsimd.affine_select(
            out=ident32[:], in_=ident32[:], compare_op=NEQ, fill=1.0,
            base=0, pattern=[[-1, 32]], channel_multiplier=1,
        )
        identP = consts.tile([128, 128], f32)
        nc.gpsimd.memset(identP[:], 0.0)
        nc.gpsimd.affine_select(
            out=identP[:], in_=identP[:], compare_op=NEQ, fill=1.0,
            base=0, pattern=[[-1, 128]], channel_multiplier=1,
        )
        # NEG * I  (fp16) for the mask inject matmul
        inegT = consts.tile([128, 128], f16)
        nc.gpsimd.memset(inegT[:], 0.0)
        nc.gpsimd.affine_select(
            out=inegT[:], in_=inegT[:], compare_op=NEQ, fill=NEG,
            base=0, pattern=[[-1, 128]], channel_multiplier=1,
        )

        # t laid out [partition, ib] so t_sb[p, ib] = t[ib*128 + p]
        t_sb = consts.tile([128, NIB], f32)
        nc.sync.dma_start(t_sb[:], t_d.rearrange("(a p) -> p a", p=128))

        # per-head transposed & scaled q/k: [32*(h%4)+d, i] packed 4 heads/tile
        qT = [consts.tile([128, S], f32r, tag=f"qT{g}", name=f"qT{g}")
              for g in range(2)]
        kT = [consts.tile([128, S], f32r, tag=f"kT{g}", name=f"kT{g}")
              for g in range(2)]
        # V in fp16, [j % 128, (jc, d)] layout; lhsT chunks vt[h][:, 32jc:32jc+32]
        vt = [consts.tile([128, NIB * D], f16, tag=f"vt{h}", name=f"vt{h}")
              for h in range(H)]

        with tc.tile_pool(name="prep", bufs=4) as prep, \
             tc.tile_pool(name="prep_ps", bufs=2, space="PSUM") as prep_ps:
            for h in range(H):
                # SWDGE dma casts f32 -> f16 inline
                nc.gpsimd.dma_start(
                    vt[h][:].rearrange("pp (jc d) -> pp jc d", d=D),
                    v_d[h].rearrange("(jc pp) d -> pp jc d", pp=128),
                )
                for (src, dstT, scaled) in ((q_d, qT, True), (k_d, kT, False)):
                    for g in range(NIB // 4):
                        ps = prep_ps.tile([32, 512], f32, tag="prep_ps")
                        for c in range(4):
                            ib = 4 * g + c
                            raw = prep.tile([128, D], f32, tag="raw")
                            nc.sync.dma_start(
                                raw[:], src[h, ib * 128:(ib + 1) * 128, :]
                            )
                            sc = prep.tile([128, D], f32, tag="sc")
                            if scaled:
                                nc.vector.tensor_scalar(
                                    sc[:], raw[:], t_sb[:, ib:ib + 1], SCALE,
                                    MULT, MULT,
                                )
                            else:
                                nc.vector.tensor_scalar(
                                    sc[:], raw[:], t_sb[:, ib:ib + 1], None, MULT,
                                )
                            nc.tensor.transpose(
                                ps[:, c * 128:(c + 1) * 128], sc[:], identP[:]
                            )
                        nc.scalar.copy(
                            dstT[h // 4][32 * (h % 4):32 * (h % 4) + 32,
                                         g * 512:(g + 1) * 512],
                            ps[:],
                        )

        # ---------------- main loop ----------------
        mpool = ctx.enter_context(tc.tile_pool(name="mask", bufs=2))
        mfpool = ctx.enter_context(tc.tile_pool(name="maskf", bufs=2))
        epool = ctx.enter_context(tc.tile_pool(name="e", bufs=2))
        ppool = ctx.enter_context(tc.tile_pool(name="p", bufs=3))
        pfpool = ctx.enter_context(tc.tile_pool(name="pf", bufs=2))
        ptpool = ctx.enter_context(tc.tile_pool(name="pts", bufs=4))
        stat = ctx.enter_context(tc.tile_pool(name="stat", bufs=8))
        opool = ctx.enter_context(tc.tile_pool(name="osb", bufs=4))
        ps_s = ctx.enter_context(tc.tile_pool(name="ps_s", bufs=2, space="PSUM"))
        ps_pt = ctx.enter_context(tc.tile_pool(name="ps_pt", bufs=2, space="PSUM"))
        ps_pv = ctx.enter_context(tc.tile_pool(name="ps_pv", bufs=2, space="PSUM"))

        for ib in range(NIB):
            i0 = ib * 128
            mask_t = mpool.tile([128, S], i32, tag="mask")
            nc.sync.dma_start(mask_t[:], m_d[i0:i0 + 128, :])
            maskf = mfpool.tile([128, S], f16, tag="maskf")
            # (1 - mask) as fp16
            nc.vector.tensor_scalar(maskf[:], mask_t[:], -1.0, 1.0, MULT, ADD)

            for h in range(H):
                hp = 32 * (h % 4)
                g = h // 4
                Sh = [ps_s.tile([128, 1024], f32, tag="S", name=f"S{half}")
                      for half in range(2)]
                # mask inject: one ldweights of NEG*I, 4 matmuls
                for half in range(2):
                    for c in range(2):
                        j0 = 1024 * half + 512 * c
                        nc.tensor.matmul(
                            Sh[half][:, 512 * c:512 * c + 512],
                            inegT[:],
                            maskf[:, j0:j0 + 512],
                            start=True, stop=False,
                        )
                # QK accumulate on top (fp32r = full PE rate at N=512)
                lhs = qT[g][hp:hp + 32, i0:i0 + 128]
                for half in range(2):
                    for c in range(2):
                        j0 = 1024 * half + 512 * c
                        nc.tensor.matmul(
                            Sh[half][:, 512 * c:512 * c + 512],
                            lhs,
                            kT[g][hp:hp + 32, j0:j0 + 512],
                            start=False, stop=True,
                            tile_position=(hp, 0),
                        )
                # exp + row-sum (masked entries come out exactly 0)
                e_t = epool.tile([128, S], f32, tag="e")
                rh = [stat.tile([128, 1], f32, tag="rh", name=f"rh{half}")
                      for half in range(2)]
                for half in range(2):
                    nc.scalar.activation(
                        e_t[:, 1024 * half:1024 * (half + 1)],
                        Sh[half][:],
                        Exp,
                        accum_out=rh[half][:],
                    )
                r_t = stat.tile([128, 1], f32, tag="r")
                nc.vector.tensor_add(r_t[:], rh[0][:], rh[1][:])
                rinv = stat.tile([128, 1], f32, tag="rinv")
                nc.vector.reciprocal(rinv[:], r_t[:])

                # normalize on GPSIMD (frees DVE/ACT for other passes)
                p_t = ppool.tile([128, S], f32, tag="p")
                nc.gpsimd.tensor_scalar(p_t[:], e_t[:], rinv[:, 0:1], None, MULT)
                nc.sync.dma_start(p_d[h, i0:i0 + 128, :], p_t[:])

                # fp16 copy of p for the PV matmul path
                pF = pfpool.tile([128, S], f16, tag="pf")
                nc.vector.tensor_copy(pF[:], p_t[:])

                # transpose p (PE, fp16) then copy PSUM->SBUF (DVE 2x)
                pT_s = []
                for half in range(2):
                    tp = ps_pt.tile([128, 1024], f16, tag="pT")
                    for kk in range(8):
                        jc = 8 * half + kk
                        nc.tensor.transpose(
                            tp[:, 128 * kk:128 * (kk + 1)],
                            pF[:, 128 * jc:128 * (jc + 1)],
                            identF[:],
                        )
                    sb = ptpool.tile([128, 1024], f16, tag="pTs")
                    nc.vector.tensor_copy(sb[:], tp[:])
                    pT_s.append(sb)

                # out^T[d, i] += V_chunk^T @ pT_chunk
                pv = ps_pv.tile([32, 128], f32, tag="pv")
                for jc in range(NIB):
                    nc.tensor.matmul(
                        pv[:],
                        vt[h][:, D * jc:D * (jc + 1)],
                        pT_s[jc // 8][:, 128 * (jc % 8):128 * (jc % 8 + 1)],
                        start=(jc == 0), stop=(jc == NIB - 1),
                    )
                oT = opool.tile([32, 128], f32, tag="oT")
                nc.scalar.copy(oT[:], pv[:])
                of = ps_pv.tile([128, 32], f32, tag="pv")
                nc.tensor.transpose(of[:], oT[:], ident32[:])
                o_s = opool.tile([128, 32], f32, tag="os")
                nc.scalar.copy(o_s[:], of[:])
                nc.scalar.dma_start(o_d[h, i0:i0 + 128, :], o_s[:])

    nc.compile()
    return nc


def _get_nc():
    if "nc" not in _cache:
        _cache["nc"] = _build_nc()
    return _cache["nc"]


def kernel(query, key, value, temp_prop_enc, mask):
    from concourse.bass_utils import run_bass_kernel_spmd

    nc = _get_nc()
    in_maps = []
    for b in range(B):
        in_maps.append({
            "q": np.ascontiguousarray(query[b], dtype=np.float32),
            "k": np.ascontiguousarray(key[b], dtype=np.float32),
            "v": np.ascontiguousarray(value[b], dtype=np.float32),
            "t": np.ascontiguousarray(temp_prop_enc[b], dtype=np.float32),
            "m": np.ascontiguousarray(mask[b, 0], dtype=np.int32),
        })
    res = run_bass_kernel_spmd(nc, in_maps, core_ids=list(range(NCORES)))
    out = np.stack([res.results[b]["o"] for b in range(B)])
    p = np.stack([res.results[b]["p"] for b in range(B)])
    return out, p
